# revision 1
# baseline (speedup 1.0000x reference)
"""Trainium2 Bass kernel for nn_MultiHeadCrossAttention (B=16, Dq=768, H=12,
hd=64, Nq=1024, Nt=64, Dkv=384) with RoPE on q and k.

Sharding: pure data-parallel over batch, 2 batches per core across 8 cores.
No collectives.

Per-core dataflow (all "T" tensors are channel-major, i.e. transposed):
  qT  = Wq.T @ feat            (PE, f32r, output stays transposed)
  qc  = qT * cos_q * scale     (DVE, fused with PSUM->SBUF move)
  qs  = qT * sin_q * scale     (DVE)
  kT  = Wk.T @ tokensT         (PE)  -> RoPE-combined into kA (=k_rot) and kB
  v   = tokens @ Wv            (PE, natural layout, duplicated across
                                partition halves so both heads of a pair
                                contract in their own array quadrant)
  scoresT = kA.T@qc + kB.T@qs  (PE, PSUM-accumulated: RoPE needs no shuffles
                                on the q side; the half-rotation is folded
                                into the k-side tensors and the table pair)
  E = exp(scoresT)             (ACT, no max-subtraction: |scores| <= ~1.3)
  D = blockdiag_ones.T @ E     (PE, all 12 head denominators into one PSUM tile)
  R = 1/D                      (DVE)
  B = indicator.T @ R          (PE, broadcasts each head's recip row to 64
                                partitions)
  E = E * B                    (DVE, normalize)
  attnT = v.T @ E              (PE)
  out = feat + Wout.T @ attnT + bias   (PE + one fused DVE op)
"""

import os
import sys
from contextlib import ExitStack

import numpy as np

sys.path.insert(0, "/opt/trn_rl_repo")

import concourse.bass as bass  # noqa: E402
import concourse.mybir as mybir  # noqa: E402
import concourse.tile as tile  # noqa: E402
from concourse import bacc  # noqa: E402
from concourse.bass_utils import run_bass_kernel_spmd  # noqa: E402

import ml_dtypes

F32 = mybir.dt.float32
BF16 = mybir.dt.bfloat16
NPBF = ml_dtypes.bfloat16

B, DQ, T, HP, WP = 16, 768, 4, 16, 16
NQ = T * HP * WP            # 1024
NT, DKV = 64, 384
H, HD = 12, 64
SCALE = HD ** -0.5
NCORES = 8
BL = B // NCORES            # batches per core = 2
CHUNK = 512                 # query positions per chunk
NCH = NQ // CHUNK           # chunks per batch = 2
KQ = DQ // 128              # 6 contraction tiles for Dq
KKV = DKV // 128            # 3 contraction tiles for Dkv
NPAIR = H // 2              # 6 head pairs


def _rope_tables(n):
    inv_freq = 1.0 / (10000.0 ** (np.arange(0, HD, 2, dtype=np.float64) / HD))
    freqs = np.arange(n, dtype=np.float64)[:, None] * inv_freq[None, :]
    emb = np.concatenate([freqs, freqs], axis=-1)  # [n, 64]
    return (np.cos(emb).T.astype(np.float32), np.sin(emb).T.astype(np.float32))


def _consts():
    cq, sq = _rope_tables(NQ)          # [64, 1024]
    ck, sk = _rope_tables(NT)          # [64, 64]
    # q tables: scale folded in, duplicated across the two heads of a pair
    cq2 = np.ascontiguousarray(np.tile(cq * SCALE, (2, 1)))       # [128, 1024]
    sq2 = np.ascontiguousarray(np.tile(sq * SCALE, (2, 1)))
    # k tables: duplicated 2 heads (partitions) x 2 batches (columns),
    # tiled KQ times along free so k-RoPE runs as one batched op
    ck2 = np.ascontiguousarray(np.tile(ck, (2, 2 * KQ)))          # [128, 768]
    sk2 = np.ascontiguousarray(np.tile(sk, (2, 2 * KQ)))
    eps = np.where(np.arange(HD) < HD // 2, -1.0, 1.0).astype(np.float32)
    epsv = np.ascontiguousarray(np.tile(eps, 2)[:, None])         # [128, 1]
    ident = np.eye(128, dtype='float32')
    # denominator lhsT: for pair j, col 2j sums partitions 0-63 (even head),
    # col 2j+1 sums partitions 64-127 (odd head)
    dlhs = np.zeros((128, NPAIR, H), np.float32)
    for j in range(NPAIR):
        dlhs[:64, j, 2 * j] = 1.0
        dlhs[64:, j, 2 * j + 1] = 1.0
    # broadcast lhsT: for pair j, row 2j feeds cols 0-63, row 2j+1 cols 64-127
    blhs = np.zeros((H, NPAIR, 128), np.float32)
    for j in range(NPAIR):
        blhs[2 * j, j, :64] = 1.0
        blhs[2 * j + 1, j, 64:] = 1.0
    import ml_dtypes as _md
    bf = _md.bfloat16
    return dict(cq=cq2, sq=sq2, ck=ck2, sk=sk2, epsv=epsv, nepsv=-epsv,
                ident=ident.astype(bf), dlhs=dlhs.astype(bf),
                blhs=blhs.astype(bf))


def _sigma_dma(nc, out_ap, in_ap):
    """out = in with 32-partition halves swapped inside each 64 block.
    On the gpsimd SWDGE ring: tiny transfers, and the sync/scalar rings
    are saturated with the front-of-kernel bulk loads."""
    for dst, src in ((0, 32), (32, 0), (64, 96), (96, 64)):
        nc.gpsimd.dma_start(out=out_ap[dst:dst + 32], in_=in_ap[src:src + 32])


def build(debug=False):
    nc = bacc.Bacc(None, target_bir_lowering=False, debug=debug)
    with tile.TileContext(nc) as tc:
        with tc.tile_pool(name="dram", bufs=1, space="DRAM") as dram:
            def din(name, shape, dt=F32):
                return dram.tile(shape, dt, kind="ExternalInput", name=name,
                                 uniquify=False)

            feat_l = din("feat_l", [BL, 128, KQ, NQ])
            feat_bf = din("feat_bf", [BL, 128, KQ, NQ], BF16)
            tok_l = din("tok_l", [BL * NT, DKV], BF16)
            wq = din("wq", [128, KQ, DQ], BF16)
            wk = din("wk", [128, KKV, DQ], BF16)
            wv = din("wv", [128, KKV, DQ], BF16)
            wout = din("wout", [128, KQ, DQ], BF16)
            bout_t = din("bout_t", [128, KQ])
            cq = din("cq", [128, NQ])
            sq = din("sq", [128, NQ])
            ck = din("ck", [128, KQ * 128])
            sk = din("sk", [128, KQ * 128])
            epsv = din("epsv", [128, 1])
            nepsv = din("nepsv", [128, 1])
            ident = din("ident", [128, 128], BF16)
            dlhs = din("dlhs", [128, NPAIR, H], BF16)
            blhs = din("blhs", [H, NPAIR, 128], BF16)
            out_l = dram.tile([BL, 128, KQ, NQ], F32, kind="ExternalOutput",
                              name="out_l", uniquify=False)

            with ExitStack() as body_ctx:
                global _body_ctx
                _body_ctx = body_ctx
                _body(nc, tc, feat_l, feat_bf, tok_l, wq, wk, wv, wout,
                      bout_t, cq, sq, ck, sk, epsv, nepsv, ident, dlhs, blhs,
                      out_l)
    nc.compile()
    return nc


def _body(nc, tc, feat_l, feat_bf, tok_l, wq, wk, wv, wout, bout_t, cq,
          sq, ck, sk, epsv, nepsv, ident, dlhs, blhs, out_l):
    MULT = mybir.AluOpType.mult
    ADD = mybir.AluOpType.add
    EXP = mybir.ActivationFunctionType.Exp

    ctx = _body_ctx
    consts = ctx.enter_context(tc.tile_pool(name="consts", bufs=1))
    kside = ctx.enter_context(tc.tile_pool(name="kside", bufs=1))
    ktmp = ctx.enter_context(tc.tile_pool(name="ktmp", bufs=1))
    featp = ctx.enter_context(tc.tile_pool(name="featp", bufs=2))
    qp = ctx.enter_context(tc.tile_pool(name="qp", bufs=2))
    ep = ctx.enter_context(tc.tile_pool(name="ep", bufs=2))
    atp = ctx.enter_context(tc.tile_pool(name="atp", bufs=2))
    outp = ctx.enter_context(tc.tile_pool(name="outp", bufs=2))
    rp = ctx.enter_context(tc.tile_pool(name="rp", bufs=2))

    pp = ctx.enter_context(tc.tile_pool(name="pp", bufs=3, space="PSUM"))
    attn = ctx.enter_context(tc.tile_pool(name="attn", bufs=4, space="PSUM"))
    dp = ctx.enter_context(tc.tile_pool(name="dp", bufs=1, space="PSUM"))

    # ---- load constants. Emission order = DGE ring order: the sync ring
    # carries the phase-0/qproj critical path, the scalar ring the bulk.
    tok_sb = consts.tile([128, DKV], BF16)
    nc.sync.dma_start(out=tok_sb, in_=tok_l[:])
    id_sb = consts.tile([128, 128], BF16)
    nc.sync.dma_start(out=id_sb, in_=ident[:])
    wk_sb = consts.tile([128, KKV, DQ], BF16)
    nc.sync.dma_start(out=wk_sb, in_=wk[:])
    wq_sb = consts.tile([128, KQ, DQ], BF16)
    nc.sync.dma_start(out=wq_sb, in_=wq[:])
    wv_sb = consts.tile([128, KKV, DQ], BF16)
    nc.scalar.dma_start(out=wv_sb, in_=wv[:])
    cq_sb = consts.tile([128, NQ], F32)
    nc.scalar.dma_start(out=cq_sb, in_=cq[:])
    sq_sb = consts.tile([128, NQ], F32)
    nc.scalar.dma_start(out=sq_sb, in_=sq[:])
    ck_sb = consts.tile([128, KQ * 128], F32)
    nc.scalar.dma_start(out=ck_sb, in_=ck[:])
    sk_sb = consts.tile([128, KQ * 128], F32)
    nc.scalar.dma_start(out=sk_sb, in_=sk[:])
    eps_sb = consts.tile([128, 1], F32)
    nc.scalar.dma_start(out=eps_sb, in_=epsv[:])
    neps_sb = consts.tile([128, 1], F32)
    nc.scalar.dma_start(out=neps_sb, in_=nepsv[:])
    dlhs_sb = consts.tile([128, NPAIR, H], BF16)
    nc.scalar.dma_start(out=dlhs_sb, in_=dlhs[:])
    blhs_sb = consts.tile([H, NPAIR, 128], BF16)
    nc.scalar.dma_start(out=blhs_sb, in_=blhs[:])
    bout_sb = consts.tile([128, KQ], F32)
    nc.scalar.dma_start(out=bout_sb, in_=bout_t[:])
    wout_sb = consts.tile([128, KQ, DQ], BF16)
    nc.scalar.dma_start(out=wout_sb, in_=wout[:])

    # ---- phase 0: tokensT, kT, k-RoPE, v ----
    _ph0 = nc.named_scope("ph0")
    _ph0.__enter__()
    tokT_sb = kside.tile([128, KKV, 128], BF16)
    for ct in range(KKV):
        tp = pp.tile([128, 128], BF16, tag="pp")
        nc.tensor.transpose(tp, tok_sb[:, ct * 128:(ct + 1) * 128], id_sb[:])
        nc.scalar.copy(out=tokT_sb[:, ct, :], in_=tp)

    kT_sb = kside.tile([128, KQ, 128], F32)
    for m in range(KQ):
        kp = pp.tile([128, 128], F32, tag="pp")
        for kc in range(KKV):
            nc.tensor.matmul(kp, wk_sb[:, kc, m * 128:(m + 1) * 128],
                             tokT_sb[:, kc, :],
                             start=(kc == 0), stop=(kc == KKV - 1))
        nc.scalar.copy(out=kT_sb[:, m, :], in_=kp)

    kA_sb = kside.tile([128, KQ, 128], BF16)
    kB_sb = kside.tile([128, KQ, 128], BF16)
    t1 = ktmp.tile([128, KQ * 128], F32, tag="t1")
    t2 = ktmp.tile([128, KQ * 128], F32, tag="t2")
    t1s = ktmp.tile([128, KQ * 128], F32, tag="t1s")
    t2s = ktmp.tile([128, KQ * 128], F32, tag="t2s")
    nc.gpsimd.tensor_mul(t1, kT_sb[:], ck_sb[:])
    nc.gpsimd.tensor_mul(t2, kT_sb[:], sk_sb[:])
    _sigma_dma(nc, t1s, t1)
    _sigma_dma(nc, t2s, t2)
    # kA = k_rot = t1 + eps * sigma(t2);  kB = t2 - eps * sigma(t1)
    nc.vector.scalar_tensor_tensor(out=kA_sb[:], in0=t2s,
                                   scalar=eps_sb[:], in1=t1,
                                   op0=MULT, op1=ADD)
    nc.vector.scalar_tensor_tensor(out=kB_sb[:], in0=t1s,
                                   scalar=neps_sb[:], in1=t2,
                                   op0=MULT, op1=ADD)

    # v, natural [token, dim] layout, duplicated across partition halves:
    # vv[0:64, b, :] == vv[64:128, b, :] == v of batch b
    vv_sb = kside.tile([128, BL, DQ], BF16)
    for b in range(BL):
        for nn0 in range(0, DQ, 512):
            nsz = min(512, DQ - nn0)
            vp = pp.tile([128, 512], F32, tag="pp")
            for half in range(2):
                for kc in range(KKV):
                    nc.tensor.matmul(
                        vp[64 * half:64 * half + 64, :nsz],
                        tokT_sb[:, kc, b * 64:(b + 1) * 64],
                        wv_sb[:, kc, nn0:nn0 + nsz],
                        start=(kc == 0), stop=(kc == KKV - 1))
            nc.scalar.copy(out=vv_sb[:, b, nn0:nn0 + nsz], in_=vp[:, :nsz])

    _ph0.__exit__(None, None, None)

    # ---- main loop: software-pipelined across the 4 (batch, chunk) steps.
    # The PE issues strictly in program order, so each chunk's serial
    # attention chains (exp -> denom -> recip -> bcast -> normalize) are
    # covered by the next chunk's dense projection matmuls; without this the
    # PE array duty cycle drops and the HAM clock-gate rethrottles to 1.2GHz.
    chunks = [(b, c) for b in range(BL) for c in range(NCH)]
    st = {}

    def stage_qproj(i):
        b, c = chunks[i]
        p0 = c * CHUNK
        featb = featp.tile([128, KQ, CHUNK], BF16, tag="featb", name=f"fb{i}")
        nc.sync.dma_start(out=featb, in_=feat_bf[b, :, :, p0:p0 + CHUNK])
        qc_sb = qp.tile([128, KQ, CHUNK], BF16, tag="qc", name=f"qc{i}")
        qs_sb = qp.tile([128, KQ, CHUNK], BF16, tag="qs", name=f"qs{i}")
        for m in range(KQ):
            qps = pp.tile([128, CHUNK], F32, tag="pp", name=f"qp{i}_{m}")
            for kc in range(KQ):
                nc.tensor.matmul(qps,
                                 wq_sb[:, kc, m * 128:(m + 1) * 128],
                                 featb[:, kc, :],
                                 start=(kc == 0), stop=(kc == KQ - 1))
            nc.vector.tensor_mul(qc_sb[:, m, :], qps, cq_sb[:, p0:p0 + CHUNK])
            nc.vector.tensor_mul(qs_sb[:, m, :], qps, sq_sb[:, p0:p0 + CHUNK])
        st[i] = dict(qc=qc_sb, qs=qs_sb)

    def stage_qk(i):
        b, c = chunks[i]
        s = st[i]
        featc = featp.tile([128, KQ, CHUNK], F32, tag="featc", name=f"fc{i}")
        nc.gpsimd.dma_start(out=featc, in_=feat_l[b, :, :, c * CHUNK:(c + 1) * CHUNK])
        s["featc"] = featc
        qc_sb, qs_sb = s["qc"], s["qs"]
        e_sb = ep.tile([128, NPAIR, CHUNK], BF16, tag="e", name=f"e{i}")
        dps = dp.tile([H, CHUNK], F32, tag="den", name=f"d{i}")

        def qk1(j):
            sps = attn.tile([128, CHUNK], F32, tag="attn", name=f"s{i}_{j}")
            for lo in range(2):  # head 2j (partitions 0:64), 2j+1 (64:128)
                o = 64 * lo
                sl = slice(o, o + 64)
                nc.tensor.matmul(sps[sl, :],
                                 kA_sb[sl, j, b * 64:(b + 1) * 64],
                                 qc_sb[sl, j, :],
                                 start=True, stop=False)
                nc.tensor.matmul(sps[sl, :],
                                 kB_sb[sl, j, b * 64:(b + 1) * 64],
                                 qs_sb[sl, j, :],
                                 start=False, stop=True)
            nc.scalar.activation(out=e_sb[:, j, :], in_=sps, func=EXP)

        def denom(j):
            nc.tensor.matmul(dps, dlhs_sb[:, j, :],
                             e_sb[:, j, :],
                             start=(j == 0), stop=(j == NPAIR - 1))

        for j in range(NPAIR):
            qk1(j)
            if j >= 1:
                denom(j - 1)
        denom(NPAIR - 1)
        s["e"], s["dps"] = e_sb, dps

    def stage_recip(i):
        s = st[i]
        r32 = rp.tile([H, CHUNK], F32, tag="r32", name=f"r32_{i}")
        nc.vector.reciprocal_approx_fast(out=r32, in_=s["dps"])
        r_sb = rp.tile([H, CHUNK], BF16, tag="r", name=f"r{i}")
        nc.scalar.copy(out=r_sb, in_=r32)
        s["r"] = r_sb

    def stage_avbc(i):
        b, c = chunks[i]
        s = st[i]
        e_sb, r_sb = s["e"], s["r"]
        attnT_sb = atp.tile([128, NPAIR, CHUNK], BF16, tag="attnT",
                            name=f"at{i}")

        def av(j):
            aps = attn.tile([128, CHUNK], F32, tag="attn", name=f"a{i}_{j}")
            for lo in range(2):
                o = 64 * lo
                sl = slice(o, o + 64)
                nc.tensor.matmul(
                    aps[sl, :],
                    vv_sb[sl, b, (2 * j + lo) * 64:(2 * j + lo + 1) * 64],
                    e_sb[sl, j, :], start=True, stop=True)
            return aps

        def bcast(j):
            bps = attn.tile([128, CHUNK], F32, tag="attn", name=f"b{i}_{j}")
            nc.tensor.matmul(bps, blhs_sb[:, j, :],
                             r_sb[:], start=True, stop=True)
            # stage to SBUF (DVE may read only one PSUM operand)
            bcs = rp.tile([128, CHUNK], F32, tag="bcs", bufs=3,
                          name=f"bc{i}_{j}")
            nc.scalar.copy(out=bcs, in_=bps)
            return bcs

        av_t, bc_t = {}, {}
        for j in range(NPAIR):
            av_t[j] = av(j)
            bc_t[j] = bcast(j)
            if j >= 1:
                nc.vector.tensor_mul(attnT_sb[:, j - 1, :],
                                     av_t[j - 1], bc_t[j - 1])
        nc.vector.tensor_mul(attnT_sb[:, NPAIR - 1, :],
                             av_t[NPAIR - 1], bc_t[NPAIR - 1])
        s["attnT"] = attnT_sb

    def stage_oproj(i):
        b, c = chunks[i]
        p0 = c * CHUNK
        s = st[i]
        attnT_sb, featc = s["attnT"], s["featc"]
        o_sb = outp.tile([128, KQ, CHUNK], F32, tag="osb", name=f"o{i}")
        for m in range(KQ):
            ops = pp.tile([128, CHUNK], F32, tag="pp", name=f"op{i}_{m}")
            for kc in range(KQ):
                nc.tensor.matmul(ops,
                                 wout_sb[:, kc, m * 128:(m + 1) * 128],
                                 attnT_sb[:, kc, :],
                                 start=(kc == 0), stop=(kc == KQ - 1))
            nc.vector.scalar_tensor_tensor(out=o_sb[:, m, :], in0=ops,
                                           scalar=bout_sb[:, m:m + 1],
                                           in1=featc[:, m, :],
                                           op0=ADD, op1=ADD)
            if m in (1, 3):
                nc.sync.dma_start(out=out_l[b, :, m - 1:m + 1, p0:p0 + CHUNK],
                                  in_=o_sb[:, m - 1:m + 1, :])
        nc.sync.dma_start(out=out_l[b, :, 4:KQ, p0:p0 + CHUNK],
                          in_=o_sb[:, 4:KQ, :])

    def scoped(fn, tag, i):
        with nc.named_scope(f"{tag}{i}"):
            fn(i)

    scoped(stage_qproj, "qp", 0)
    scoped(stage_qk, "qk", 0)
    n = len(chunks)
    for i in range(n):
        scoped(stage_recip, "rc", i)
        if i + 1 < n:
            scoped(stage_qproj, "qp", i + 1)
        scoped(stage_avbc, "av", i)
        if i + 1 < n:
            scoped(stage_qk, "qk", i + 1)
        scoped(stage_oproj, "op", i)


_NC_CACHE = {}


def _get_nc():
    if "nc" not in _NC_CACHE:
        _NC_CACHE["nc"] = build(debug=False)
    return _NC_CACHE["nc"]


def _prep_in_maps(feat, tokens, Wq, Wkv, Wout, bout):
    feat = np.ascontiguousarray(feat, dtype=np.float32).reshape(B, DQ, NQ)
    tokens = np.ascontiguousarray(tokens, dtype=np.float32)
    shared = dict(
        wq=np.ascontiguousarray(
            Wq.reshape(KQ, 128, DQ).transpose(1, 0, 2), dtype=NPBF),
        wk=np.ascontiguousarray(
            Wkv[:, :DQ].reshape(KKV, 128, DQ).transpose(1, 0, 2), dtype=NPBF),
        wv=np.ascontiguousarray(
            Wkv[:, DQ:].reshape(KKV, 128, DQ).transpose(1, 0, 2), dtype=NPBF),
        wout=np.ascontiguousarray(
            Wout.reshape(KQ, 128, DQ).transpose(1, 0, 2), dtype=NPBF),
        bout_t=np.ascontiguousarray(bout.reshape(KQ, 128).T, dtype=np.float32),
        **_consts(),
    )
    in_maps = []
    for cid in range(NCORES):
        sl = slice(BL * cid, BL * (cid + 1))
        fl = np.ascontiguousarray(
            feat[sl].reshape(BL, KQ, 128, NQ).transpose(0, 2, 1, 3))
        tl = np.ascontiguousarray(tokens[sl].reshape(BL * NT, DKV), dtype=NPBF)
        in_maps.append(dict(feat_l=fl, feat_bf=fl.astype(NPBF), tok_l=tl,
                            **shared))
    return in_maps


def _install_ntff_hook():
    """The container's antenv lacks axon_hooks; register the NTFF profile
    hook from trn_agent_boot ourselves so trace=True yields HW exec times."""
    import types

    import antenv
    from trn_agent_boot.trn_boot import _ntff_profile_via_ctypes

    mod = types.ModuleType("antenv.axon_hooks")
    state = {"hook": None}
    mod.set_axon_ntff_profile_hook = lambda h: state.__setitem__("hook", h)
    mod.get_axon_ntff_profile_hook = lambda: state["hook"]
    sys.modules["antenv.axon_hooks"] = mod
    antenv.axon_hooks = mod
    mod.set_axon_ntff_profile_hook(
        _ntff_profile_via_ctypes("/opt/axon/libaxon_pjrt.so"))
    # the S3 artifact upload has no credentials here; make it a no-op
    import concourse.bass_utils as bu
    bu.upload_artifacts = lambda tmpdir: f"local:{tmpdir}"


def run(inputs, trace=False, trace_cores=None):
    nc = _get_nc()
    if trace:
        try:
            _install_ntff_hook()
        except Exception as e:  # profiling is best-effort
            print(f"ntff hook install failed: {e}", file=sys.stderr)
            trace = False
    in_maps = _prep_in_maps(**inputs)
    res = run_bass_kernel_spmd(nc, in_maps, core_ids=list(range(NCORES)),
                               trace=trace, trace_cores=trace_cores)
    outs = []
    for r in res.results:
        ol = r["out_l"]  # [BL, 128, KQ, NQ]
        outs.append(ol.transpose(0, 2, 1, 3).reshape(BL, DQ, T, HP, WP))
    return np.ascontiguousarray(np.concatenate(outs, axis=0)), res


def kernel(**inputs):
    return run(inputs, trace=False)[0]



# revision 2
# speedup vs baseline: 1.0794x; 1.0794x over previous
"""Trainium2 Bass kernel for nn_MultiHeadCrossAttention (B=16, Dq=768, H=12,
hd=64, Nq=1024, Nt=64, Dkv=384) with RoPE on q and k.

Sharding: pure data-parallel over batch, 2 batches per core across 8 cores.

v3: super-chunk schedule — the two 512-query chunks of a batch run
back-to-back through every weight-stationary stage so each LDWEIGHTS is
amortized over two matmuls.  fp8 DoubleRow for qproj/denominator/oproj,
bf16 block-diagonal head-pair matmuls for scores, fp8 for AV.  Residual
add folded into the out-proj PSUM group via a 1024*I identity matmul on
bf16 feat (bout folded into feat host-side).  qc/qs are produced by one
broadcast tensor_mul per m-tile against a combined [cos;sin] table.

Scaling scheme (fp8 e4m3 range, max +-240):
  weights *32 host-side; qstage = 32*q (bf16); qc/qs = qstage*(tab*SCALE)
  kA/kB = 32*k_rot bf16; scores PSUM = 1024*true -> exp(x/1024)
  e = exp(true) fp8; vv = 32*v fp8; av PSUM = 32*attn_un; bcs = 1/D
  attnT = 32*attnT_true fp8; oproj PSUM = 1024*out -> ACT stage * 1/1024
"""

import os
import sys
from contextlib import ExitStack

import numpy as np

sys.path.insert(0, "/opt/trn_rl_repo")

import concourse.bass as bass  # noqa: E402
import concourse.mybir as mybir  # noqa: E402
import concourse.tile as tile  # noqa: E402
from concourse import bacc  # noqa: E402
from concourse.bass_utils import run_bass_kernel_spmd  # noqa: E402

import ml_dtypes

F32 = mybir.dt.float32
BF16 = mybir.dt.bfloat16
FP8 = mybir.dt.float8e4
I32 = mybir.dt.int32
NPBF = ml_dtypes.bfloat16
NPF8 = ml_dtypes.float8_e4m3
DR = mybir.MatmulPerfMode.DoubleRow

B, DQ, T, HP, WP = 16, 768, 4, 16, 16
NQ = T * HP * WP            # 1024
NT, DKV = 64, 384
H, HD = 12, 64
SCALE = HD ** -0.5
NCORES = 8
BL = B // NCORES            # batches per core = 2
CHUNK = 512
NCH = NQ // CHUNK           # chunks per batch = 2
KQ = DQ // 128
KKV = DKV // 128
NPAIR = H // 2
WS = 32.0
PSCALE = 1.0 / (WS * WS)


def _rope_tables(n):
    inv_freq = 1.0 / (10000.0 ** (np.arange(0, HD, 2, dtype=np.float64) / HD))
    freqs = np.arange(n, dtype=np.float64)[:, None] * inv_freq[None, :]
    emb = np.concatenate([freqs, freqs], axis=-1)  # [n, 64]
    return (np.cos(emb).T.astype(np.float32), np.sin(emb).T.astype(np.float32))


def _consts():
    cq, sq = _rope_tables(NQ)          # [64, 1024]
    ck, sk = _rope_tables(NT)          # [64, 64]
    # combined q table: [128, {cos,sin}, NQ], attention scale folded in,
    # duplicated across the two heads of a pair (partition halves)
    cqs = np.stack([np.tile(cq * SCALE, (2, 1)),
                    np.tile(sq * SCALE, (2, 1))], axis=1)  # [128, 2, 1024]
    ck2 = np.ascontiguousarray(np.tile(ck, (2, 2 * KQ)))   # [128, 768]
    sk2 = np.ascontiguousarray(np.tile(sk, (2, 2 * KQ)))
    eps = np.where(np.arange(HD) < HD // 2, -1.0, 1.0).astype(np.float32)
    epsv = np.ascontiguousarray(np.tile(eps, 2)[:, None])  # [128, 1]
    ident = np.eye(128, dtype='float32')
    # denominator lhsT (fp8, 16 cols for the DoubleRow stride rule):
    # pad cols 12-15 get a row-0 one so their denominators aren't 0.
    dlhs = np.zeros((128, NPAIR, 16), np.float32)
    for j in range(NPAIR):
        dlhs[:64, j, 2 * j] = 1.0
        dlhs[64:, j, 2 * j + 1] = 1.0
    dlhs[0, :, 12:] = 1.0
    blhs = np.zeros((H, NPAIR, 128), np.float32)
    for j in range(NPAIR):
        blhs[2 * j, j, :64] = 1.0
        blhs[2 * j + 1, j, 64:] = 1.0
    return dict(cqs=np.ascontiguousarray(cqs).astype(NPBF),
                ck=ck2.astype(NPBF), sk=sk2.astype(NPBF),
                epsv=epsv, nepsv=-epsv,
                ident=ident.astype(NPBF),
                id1024=(ident * 1024.0).astype(NPBF),
                dlhs=dlhs.astype(NPF8),
                blhs=blhs.astype(NPBF))


def _sigma_dma(nc, out_ap, in_ap):
    """out = in with 32-partition halves swapped inside each 64 block."""
    for dst, src in ((0, 32), (32, 0), (64, 96), (96, 64)):
        nc.gpsimd.dma_start(out=out_ap[dst:dst + 32], in_=in_ap[src:src + 32])


def build(debug=False):
    nc = bacc.Bacc(None, target_bir_lowering=False, debug=debug)
    with tile.TileContext(nc) as tc:
        with tc.tile_pool(name="dram", bufs=1, space="DRAM") as dram:
            def din(name, shape, dt=F32):
                return dram.tile(shape, dt, kind="ExternalInput", name=name,
                                 uniquify=False)

            feat8 = din("feat8", [BL, 128, KQ, NQ], FP8)
            featr = din("featr", [BL, 128, KQ, NQ], BF16)
            tok_l = din("tok_l", [BL * NT, DKV], BF16)
            wq = din("wq", [128, KQ, DQ], FP8)
            wk = din("wk", [128, KKV, DQ], FP8)
            wv = din("wv", [128, KKV, DQ], FP8)
            wout = din("wout", [128, KQ, DQ], FP8)
            cqs = din("cqs", [128, 2, NQ], BF16)
            ck = din("ck", [128, KQ * 128], BF16)
            sk = din("sk", [128, KQ * 128], BF16)
            epsv = din("epsv", [128, 1])
            nepsv = din("nepsv", [128, 1])
            ident = din("ident", [128, 128], BF16)
            id1024 = din("id1024", [128, 128], BF16)
            dlhs = din("dlhs", [128, NPAIR, 16], FP8)
            blhs = din("blhs", [H, NPAIR, 128], BF16)
            out_l = dram.tile([BL, 128, KQ, NQ], F32, kind="ExternalOutput",
                              name="out_l", uniquify=False)

            with ExitStack() as body_ctx:
                global _body_ctx
                _body_ctx = body_ctx
                _body(nc, tc, feat8, featr, tok_l, wq, wk, wv, wout,
                      cqs, ck, sk, epsv, nepsv, ident, id1024, dlhs, blhs,
                      out_l)
    nc.compile()
    return nc


def _body(nc, tc, feat8, featr, tok_l, wq, wk, wv, wout, cqs, ck, sk,
          epsv, nepsv, ident, id1024, dlhs, blhs, out_l):
    MULT = mybir.AluOpType.mult
    ADD = mybir.AluOpType.add
    EXP = mybir.ActivationFunctionType.Exp
    COPY = mybir.ActivationFunctionType.Copy

    ctx = _body_ctx
    consts = ctx.enter_context(tc.tile_pool(name="consts", bufs=1))
    kside = ctx.enter_context(tc.tile_pool(name="kside", bufs=1))
    ktmp = ctx.enter_context(tc.tile_pool(name="ktmp", bufs=1))
    featp = ctx.enter_context(tc.tile_pool(name="featp", bufs=2))
    qp = ctx.enter_context(tc.tile_pool(name="qp", bufs=1))
    qsp = ctx.enter_context(tc.tile_pool(name="qsp", bufs=2))
    ep = ctx.enter_context(tc.tile_pool(name="ep", bufs=2))
    atp = ctx.enter_context(tc.tile_pool(name="atp", bufs=2))
    outp = ctx.enter_context(tc.tile_pool(name="outp", bufs=1))
    rp = ctx.enter_context(tc.tile_pool(name="rp", bufs=2))

    pp = ctx.enter_context(tc.tile_pool(name="pp", bufs=2, space="PSUM"))
    attn = ctx.enter_context(tc.tile_pool(name="attn", bufs=4, space="PSUM"))
    dp = ctx.enter_context(tc.tile_pool(name="dp", bufs=2, space="PSUM"))

    st = {}

    # block-diagonalized RoPE'd k: [128, pair, batch, {kA,kB}, 128] bf16
    kbd = kside.tile([128, NPAIR, BL, 2, 128], BF16)
    # block-diagonalized v: [128, batch, pair, 128] fp8
    vbd = kside.tile([128, BL, NPAIR, 128], FP8)
    # zero the off-diagonal quadrants (cheap int32-bitcast DVE memsets)
    nc.vector.memset(kbd[:].bitcast(I32), 0)
    nc.vector.memset(vbd[:].bitcast(I32), 0)

    # ---- constant DMAs.  sync: q-side critical path; scalar: k-side
    # weights + small consts; gpsimd: k tables (sigma DMAs follow there).
    tok_sb = consts.tile([128, DKV], BF16)
    nc.sync.dma_start(out=tok_sb, in_=tok_l[:])
    id_sb = consts.tile([128, 128], BF16)
    nc.sync.dma_start(out=id_sb, in_=ident[:])
    wq_sb = consts.tile([128, KQ, DQ], FP8)
    nc.sync.dma_start(out=wq_sb, in_=wq[:])
    featb0 = featp.tile([128, KQ, NQ], FP8, tag="featb", name="fb0")
    nc.sync.dma_start(out=featb0, in_=feat8[0, :, :, :])
    st.setdefault(0, {})["featb"] = featb0
    cqs_sb = consts.tile([128, 2, NQ], BF16)
    nc.sync.dma_start(out=cqs_sb, in_=cqs[:])
    blhs_sb = consts.tile([H, NPAIR, 128], BF16)
    nc.sync.dma_start(out=blhs_sb, in_=blhs[:])
    wout_sb = consts.tile([128, KQ, DQ], FP8)
    nc.sync.dma_start(out=wout_sb, in_=wout[:])
    id1024_sb = consts.tile([128, 128], BF16)
    nc.sync.dma_start(out=id1024_sb, in_=id1024[:])

    wk_sb = consts.tile([128, KKV, DQ], FP8)
    nc.scalar.dma_start(out=wk_sb, in_=wk[:])
    wv_sb = consts.tile([128, KKV, DQ], FP8)
    nc.scalar.dma_start(out=wv_sb, in_=wv[:])
    eps_sb = consts.tile([128, 1], F32)
    nc.scalar.dma_start(out=eps_sb, in_=epsv[:])
    neps_sb = consts.tile([128, 1], F32)
    nc.scalar.dma_start(out=neps_sb, in_=nepsv[:])
    dlhs_sb = consts.tile([128, NPAIR, 16], FP8)
    nc.scalar.dma_start(out=dlhs_sb, in_=dlhs[:])

    ck_sb = consts.tile([128, KQ * 128], BF16)
    nc.gpsimd.dma_start(out=ck_sb, in_=ck[:])
    sk_sb = consts.tile([128, KQ * 128], BF16)
    nc.gpsimd.dma_start(out=sk_sb, in_=sk[:])

    # ---- phase 0a: tokensT + kT (earliest possible PE work)
    _s = nc.named_scope("ph0a")
    _s.__enter__()
    tokT_sb = kside.tile([128, KKV, 128], FP8)
    for ct in range(KKV):
        tp = pp.tile([128, 128], BF16, tag="pp")
        nc.tensor.transpose(tp, tok_sb[:, ct * 128:(ct + 1) * 128], id_sb[:])
        nc.scalar.copy(out=tokT_sb[:, ct, :], in_=tp)
    kT_sb = kside.tile([128, KQ, 128], F32)
    for m in range(KQ):
        kp = pp.tile([128, 128], F32, tag="pp")
        for kc in range(KKV):
            nc.tensor.matmul(kp, wk_sb[:, kc, m * 128:(m + 1) * 128],
                             tokT_sb[:, kc, :],
                             start=(kc == 0), stop=(kc == KKV - 1))
        nc.scalar.copy(out=kT_sb[:, m, :], in_=kp)
    _s.__exit__(None, None, None)

    def phase0_krope():
        t1 = ktmp.tile([128, KQ * 128], F32, tag="t1")
        t2 = ktmp.tile([128, KQ * 128], F32, tag="t2")
        t1s = ktmp.tile([128, KQ * 128], F32, tag="t1s")
        t2s = ktmp.tile([128, KQ * 128], F32, tag="t2s")
        nc.vector.tensor_mul(t1, kT_sb[:], ck_sb[:])
        nc.vector.tensor_mul(t2, kT_sb[:], sk_sb[:])
        _sigma_dma(nc, t1s, t1)
        _sigma_dma(nc, t2s, t2)
        for lo in range(2):
            sl = slice(64 * lo, 64 * lo + 64)
            cl = slice(64 * lo, 64 * lo + 64)
            v3 = lambda t: t[sl, :].rearrange(
                "p (j b t) -> p j b t", j=NPAIR, b=BL)
            nc.vector.scalar_tensor_tensor(
                out=kbd[sl, :, :, 0, cl], in0=v3(t2s),
                scalar=eps_sb[sl], in1=v3(t1), op0=MULT, op1=ADD)
            nc.vector.scalar_tensor_tensor(
                out=kbd[sl, :, :, 1, cl], in0=v3(t1s),
                scalar=neps_sb[sl], in1=v3(t2), op0=MULT, op1=ADD)

    def phase0_v():
        for b in range(BL):
            for (nn0, nsz) in ((0, 512), (512, 256)):
                vp = pp.tile([128, 512], F32, tag="pp")
                for half in range(2):
                    for kc in range(KKV):
                        nc.tensor.matmul(
                            vp[64 * half:64 * half + 64, :nsz],
                            tokT_sb[:, kc, b * 64:(b + 1) * 64],
                            wv_sb[:, kc, nn0:nn0 + nsz],
                            start=(kc == 0), stop=(kc == KKV - 1))
                npair_blk = nsz // 128
                j0 = nn0 // 128
                for half in range(2):
                    sl = slice(64 * half, 64 * half + 64)
                    cl = slice(64 * half, 64 * half + 64)
                    nc.scalar.copy(
                        out=vbd[sl, b, j0:j0 + npair_blk, cl],
                        in_=vp[sl, :nsz].rearrange(
                            "p (j two d) -> p j two d", two=2, d=64
                        )[:, :, half, :])

    # ---- super-chunk stages.  b = batch; each runs chunks c=0,1 paired.

    def qproj(b, ms):
        """qproj m-tiles in `ms` for both chunks of batch b."""
        s = st.setdefault(b, {})
        if "q" not in s:
            if "featb" not in s:
                featb = featp.tile([128, KQ, NQ], FP8, tag="featb",
                                   name=f"fb{b}")
                nc.sync.dma_start(out=featb, in_=feat8[b, :, :, :])
                s["featb"] = featb
            s["q"] = qp.tile([128, KQ, 2, NQ], BF16, tag="q", name=f"q{b}")
        featb, q_sb = s["featb"], s["q"]
        for m in ms:
            qps = {}
            for c in range(NCH):
                qps[c] = pp.tile([128, CHUNK], F32, tag="pp",
                                 name=f"qp{b}_{m}_{c}")
            for kk in range(KQ // 2):
                for c in range(NCH):
                    nc.tensor.matmul(
                        qps[c],
                        wq_sb[:, 2 * kk:2 * kk + 2, m * 128:(m + 1) * 128],
                        featb[:, 2 * kk:2 * kk + 2,
                              c * CHUNK:(c + 1) * CHUNK],
                        start=(kk == 0), stop=(kk == KQ // 2 - 1),
                        perf_mode=DR)
            for c in range(NCH):
                p0 = c * CHUNK
                qstage = qsp.tile([128, CHUNK], BF16, tag="qs", bufs=3,
                                  name=f"qst{b}_{m}_{c}")
                nc.scalar.copy(out=qstage, in_=qps[c])
                nc.vector.tensor_mul(
                    q_sb[:, m, :, p0:p0 + CHUNK],
                    qstage[:, :].unsqueeze(1).broadcast_to([128, 2, CHUNK]),
                    cqs_sb[:, :, p0:p0 + CHUNK])

    def scores(b, js):
        s = st[b]
        q_sb = s["q"]
        if "e" not in s:
            s["e"] = [ep.tile([128, NPAIR, CHUNK], FP8, tag="e",
                              name=f"e{b}_{c}") for c in range(NCH)]
            s["sps"] = {}
        for j in js:
            for c in range(NCH):
                s["sps"][(j, c)] = attn.tile([128, CHUNK], F32, tag="attn",
                                             name=f"s{b}_{j}_{c}")
            for ab in range(2):  # kA then kB (accumulate)
                for c in range(NCH):
                    nc.tensor.matmul(
                        s["sps"][(j, c)], kbd[:, j, b, ab, :],
                        q_sb[:, j, ab, c * CHUNK:(c + 1) * CHUNK],
                        start=(ab == 0), stop=(ab == 1))
            for c in range(NCH):
                nc.scalar.activation(out=s["e"][c][:, j, :],
                                     in_=s["sps"][(j, c)], func=EXP,
                                     scale=PSCALE)

    def denom(b, u):
        s = st[b]
        if "dps" not in s:
            s["dps"] = [dp.tile([16, CHUNK], F32, tag="den",
                                name=f"d{b}_{c}") for c in range(NCH)]
        for c in range(NCH):
            nc.tensor.matmul(s["dps"][c], dlhs_sb[:, 2 * u:2 * u + 2, :],
                             s["e"][c][:, 2 * u:2 * u + 2, :],
                             start=(u == 0), stop=(u == NPAIR // 2 - 1),
                             perf_mode=DR)

    def recip(b):
        s = st[b]
        s["r"] = []
        for c in range(NCH):
            r32 = rp.tile([16, CHUNK], F32, tag="r32", name=f"r32_{b}_{c}")
            nc.vector.reciprocal_approx_fast(out=r32, in_=s["dps"][c])
            r_sb = rp.tile([H, CHUNK], BF16, tag="r", name=f"r{b}_{c}")
            nc.scalar.copy(out=r_sb, in_=r32[:H, :])
            s["r"].append(r_sb)

    def avbc(b, js):
        s = st[b]
        if "attnT" not in s:
            s["attnT"] = [atp.tile([128, NPAIR, CHUNK], FP8, tag="attnT",
                                   name=f"at{b}_{c}") for c in range(NCH)]
        for j in js:
            aps, bps = {}, {}
            for c in range(NCH):
                aps[c] = attn.tile([128, CHUNK], F32, tag="attn",
                                   name=f"a{b}_{j}_{c}")
                nc.tensor.matmul(aps[c], vbd[:, b, j, :], s["e"][c][:, j, :],
                                 start=True, stop=True)
            for c in range(NCH):
                bps[c] = attn.tile([128, CHUNK], F32, tag="attn",
                                   name=f"b{b}_{j}_{c}")
                nc.tensor.matmul(bps[c], blhs_sb[:, j, :], s["r"][c][:],
                                 start=True, stop=True)
            for c in range(NCH):
                avs = rp.tile([128, CHUNK], BF16, tag="avs", bufs=4,
                              name=f"avs{b}_{j}_{c}")
                nc.scalar.copy(out=avs, in_=aps[c])
                nc.vector.tensor_mul(s["attnT"][c][:, j, :], avs, bps[c])

    def oproj(b, ms, den_next=None):
        s = st[b]
        featrb, attnT = s["featrb"], s["attnT"]
        if "o" not in s:
            s["o"] = outp.tile([128, KQ, NQ], F32, tag="osb", name=f"o{b}")
        o_sb = s["o"]
        for mi, m in enumerate(ms):
            ops = {}
            for c in range(NCH):
                ops[c] = pp.tile([128, CHUNK], F32, tag="pp",
                                 name=f"op{b}_{m}_{c}")
            for kk in range(KQ // 2):
                for c in range(NCH):
                    nc.tensor.matmul(
                        ops[c],
                        wout_sb[:, 2 * kk:2 * kk + 2, m * 128:(m + 1) * 128],
                        attnT[c][:, 2 * kk:2 * kk + 2, :],
                        start=(kk == 0), stop=False, perf_mode=DR)
            for c in range(NCH):
                nc.tensor.matmul(ops[c], id1024_sb[:],
                                 featrb[:, m, c * CHUNK:(c + 1) * CHUNK],
                                 start=False, stop=True)
            if den_next is not None and m % 2 == 1:
                denom(den_next, m // 2)
            for c in range(NCH):
                nc.scalar.activation(out=o_sb[:, m, c * CHUNK:(c + 1) * CHUNK],
                                     in_=ops[c], func=COPY, scale=PSCALE)
            if m in (1, 3, 5):
                nc.sync.dma_start(out=out_l[b, :, m - 1:m + 1, :],
                                  in_=o_sb[:, m - 1:m + 1, :])

    def load_featr(b):
        s = st.setdefault(b, {})
        featrb = featp.tile([128, KQ, NQ], BF16, tag="featr", name=f"fr{b}")
        nc.scalar.dma_start(out=featrb, in_=featr[b, :, :, :])
        s["featrb"] = featrb

    def sc(tag):
        return nc.named_scope(tag)

    # ---- emission schedule (PE program order)
    with sc("ph0k"):
        phase0_krope()
    with sc("qp0"):
        load_featr(0)
        qproj(0, range(KQ))
    with sc("ph0v"):
        phase0_v()
    with sc("qk0"):
        scores(0, range(NPAIR))
    with sc("dn0"):
        for u in range(NPAIR // 2):
            denom(0, u)
    with sc("rc0"):
        recip(0)
    with sc("qp1"):
        load_featr(1)
        qproj(1, range(KQ))
    with sc("av0"):
        avbc(0, range(NPAIR))
    with sc("qk1"):
        scores(1, range(NPAIR))
    with sc("op0"):
        oproj(0, range(KQ), den_next=1)
    with sc("rc1"):
        recip(1)
    with sc("av1"):
        avbc(1, range(NPAIR))
    with sc("op1"):
        oproj(1, range(KQ))


_NC_CACHE = {}


def _get_nc():
    if "nc" not in _NC_CACHE:
        _NC_CACHE["nc"] = build(debug=False)
    return _NC_CACHE["nc"]


def _prep_in_maps(feat, tokens, Wq, Wkv, Wout, bout):
    feat = np.ascontiguousarray(feat, dtype=np.float32).reshape(B, DQ, NQ)
    tokens = np.ascontiguousarray(tokens, dtype=np.float32)
    featb = feat + bout.astype(np.float32)[None, :, None]
    shared = dict(
        wq=np.ascontiguousarray(
            (Wq * WS).reshape(KQ, 128, DQ).transpose(1, 0, 2), dtype=NPF8),
        wk=np.ascontiguousarray(
            (Wkv[:, :DQ] * WS).reshape(KKV, 128, DQ).transpose(1, 0, 2),
            dtype=NPF8),
        wv=np.ascontiguousarray(
            (Wkv[:, DQ:] * WS).reshape(KKV, 128, DQ).transpose(1, 0, 2),
            dtype=NPF8),
        wout=np.ascontiguousarray(
            (Wout * WS).reshape(KQ, 128, DQ).transpose(1, 0, 2), dtype=NPF8),
        **_consts(),
    )
    in_maps = []
    for cid in range(NCORES):
        sl = slice(BL * cid, BL * (cid + 1))
        fl = feat[sl].reshape(BL, KQ, 128, NQ).transpose(0, 2, 1, 3)
        fr = featb[sl].reshape(BL, KQ, 128, NQ).transpose(0, 2, 1, 3)
        tl = np.ascontiguousarray(tokens[sl].reshape(BL * NT, DKV), dtype=NPBF)
        in_maps.append(dict(
            feat8=np.ascontiguousarray(fl, dtype=NPF8),
            featr=np.ascontiguousarray(fr, dtype=NPBF),
            tok_l=tl, **shared))
    return in_maps


def _install_ntff_hook():
    import types

    import antenv
    from trn_agent_boot.trn_boot import _ntff_profile_via_ctypes

    mod = types.ModuleType("antenv.axon_hooks")
    state = {"hook": None}
    mod.set_axon_ntff_profile_hook = lambda h: state.__setitem__("hook", h)
    mod.get_axon_ntff_profile_hook = lambda: state["hook"]
    sys.modules["antenv.axon_hooks"] = mod
    antenv.axon_hooks = mod
    mod.set_axon_ntff_profile_hook(
        _ntff_profile_via_ctypes("/opt/axon/libaxon_pjrt.so"))
    import concourse.bass_utils as bu
    bu.upload_artifacts = lambda tmpdir: f"local:{tmpdir}"


def run(inputs, trace=False, trace_cores=None):
    nc = _get_nc()
    if trace:
        try:
            _install_ntff_hook()
        except Exception as e:
            print(f"ntff hook install failed: {e}", file=sys.stderr)
            trace = False
    in_maps = _prep_in_maps(**inputs)
    res = run_bass_kernel_spmd(nc, in_maps, core_ids=list(range(NCORES)),
                               trace=trace, trace_cores=trace_cores)
    outs = []
    for r in res.results:
        ol = r["out_l"]  # [BL, 128, KQ, NQ]
        outs.append(ol.transpose(0, 2, 1, 3).reshape(BL, DQ, T, HP, WP))
    return np.ascontiguousarray(np.concatenate(outs, axis=0)), res


def kernel(**inputs):
    return run(inputs, trace=False)[0]


# revision 3
# speedup vs baseline: 1.1041x; 1.0229x over previous
"""Trainium2 Bass kernel for nn_MultiHeadCrossAttention (B=16, Dq=768, H=12,
hd=64, Nq=1024, Nt=64, Dkv=384) with RoPE on q and k.

Sharding: pure data-parallel over batch, 2 batches per core across 8 cores.

v3: super-chunk schedule — the two 512-query chunks of a batch run
back-to-back through every weight-stationary stage so each LDWEIGHTS is
amortized over two matmuls.  fp8 DoubleRow for qproj/denominator/oproj,
bf16 block-diagonal head-pair matmuls for scores, fp8 for AV.  Residual
add folded into the out-proj PSUM group via a 1024*I identity matmul on
bf16 feat (bout folded into feat host-side).  qc/qs are produced by one
broadcast tensor_mul per m-tile against a combined [cos;sin] table.

Scaling scheme (fp8 e4m3 range, max +-240):
  weights *32 host-side; qstage = 32*q (bf16); qc/qs = qstage*(tab*SCALE)
  kA/kB = 32*k_rot bf16; scores PSUM = 1024*true -> exp(x/1024)
  e = exp(true) fp8; vv = 32*v fp8; av PSUM = 32*attn_un; bcs = 1/D
  attnT = 32*attnT_true fp8; oproj PSUM = 1024*out -> ACT stage * 1/1024
"""

import os
import sys
from contextlib import ExitStack

import numpy as np

sys.path.insert(0, "/opt/trn_rl_repo")

import concourse.bass as bass  # noqa: E402
import concourse.mybir as mybir  # noqa: E402
import concourse.tile as tile  # noqa: E402
from concourse import bacc  # noqa: E402
from concourse.bass_utils import run_bass_kernel_spmd  # noqa: E402

import ml_dtypes

F32 = mybir.dt.float32
BF16 = mybir.dt.bfloat16
FP8 = mybir.dt.float8e4
I32 = mybir.dt.int32
NPBF = ml_dtypes.bfloat16
NPF8 = ml_dtypes.float8_e4m3
DR = mybir.MatmulPerfMode.DoubleRow

B, DQ, T, HP, WP = 16, 768, 4, 16, 16
NQ = T * HP * WP            # 1024
NT, DKV = 64, 384
H, HD = 12, 64
SCALE = HD ** -0.5
NCORES = 8
BL = B // NCORES            # batches per core = 2
CHUNK = 512
NCH = NQ // CHUNK           # chunks per batch = 2
KQ = DQ // 128
KKV = DKV // 128
NPAIR = H // 2
WS = 32.0
PSCALE = 1.0 / (WS * WS)


def _rope_tables(n):
    inv_freq = 1.0 / (10000.0 ** (np.arange(0, HD, 2, dtype=np.float64) / HD))
    freqs = np.arange(n, dtype=np.float64)[:, None] * inv_freq[None, :]
    emb = np.concatenate([freqs, freqs], axis=-1)  # [n, 64]
    return (np.cos(emb).T.astype(np.float32), np.sin(emb).T.astype(np.float32))


def _consts():
    cq, sq = _rope_tables(NQ)          # [64, 1024]
    ck, sk = _rope_tables(NT)          # [64, 64]
    # combined q table: [128, {cos,sin}, NQ], attention scale folded in,
    # duplicated across the two heads of a pair (partition halves)
    cqs = np.stack([np.tile(cq * SCALE, (2, 1)),
                    np.tile(sq * SCALE, (2, 1))], axis=1)  # [128, 2, 1024]
    ck2 = np.ascontiguousarray(np.tile(ck, (2, 2 * KQ)))   # [128, 768]
    sk2 = np.ascontiguousarray(np.tile(sk, (2, 2 * KQ)))
    eps = np.where(np.arange(HD) < HD // 2, -1.0, 1.0).astype(np.float32)
    epsv = np.ascontiguousarray(np.tile(eps, 2)[:, None])  # [128, 1]
    ident = np.eye(128, dtype='float32')
    # denominator lhsT (fp8, 16 cols for the DoubleRow stride rule):
    # pad cols 12-15 get a row-0 one so their denominators aren't 0.
    dlhs = np.zeros((128, NPAIR, 16), np.float32)
    for j in range(NPAIR):
        dlhs[:64, j, 2 * j] = 1.0
        dlhs[64:, j, 2 * j + 1] = 1.0
    dlhs[0, :, 12:] = 1.0
    blhs = np.zeros((H, NPAIR, 128), np.float32)
    for j in range(NPAIR):
        blhs[2 * j, j, :64] = 1.0
        blhs[2 * j + 1, j, 64:] = 1.0
    return dict(cqs=np.ascontiguousarray(cqs).astype(NPBF),
                ck=ck2.astype(NPBF), sk=sk2.astype(NPBF),
                epsv=epsv, nepsv=-epsv,
                ident=ident.astype(NPBF),
                id1024=(ident * 1024.0).astype(NPBF),
                dlhs=dlhs.astype(NPF8),
                blhs=blhs.astype(NPBF))


def _sigma_dma(nc, out_ap, in_ap):
    """out = in with 32-partition halves swapped inside each 64 block."""
    for dst, src in ((0, 32), (32, 0), (64, 96), (96, 64)):
        nc.gpsimd.dma_start(out=out_ap[dst:dst + 32], in_=in_ap[src:src + 32])


def build(debug=False):
    nc = bacc.Bacc(None, target_bir_lowering=False, debug=debug)
    with tile.TileContext(nc) as tc:
        with tc.tile_pool(name="dram", bufs=1, space="DRAM") as dram:
            def din(name, shape, dt=F32):
                return dram.tile(shape, dt, kind="ExternalInput", name=name,
                                 uniquify=False)

            feat8 = din("feat8", [BL, 128, KQ, NQ], FP8)
            featr = din("featr", [BL, 128, KQ, NQ], BF16)
            tok_l = din("tok_l", [BL * NT, DKV], BF16)
            wq = din("wq", [128, KQ, DQ], FP8)
            wk = din("wk", [128, KKV, DQ], FP8)
            wv = din("wv", [128, KKV, DQ], FP8)
            wout = din("wout", [128, KQ, DQ], FP8)
            cqs = din("cqs", [128, 2, NQ], BF16)
            ck = din("ck", [128, KQ * 128], BF16)
            sk = din("sk", [128, KQ * 128], BF16)
            epsv = din("epsv", [128, 1])
            nepsv = din("nepsv", [128, 1])
            ident = din("ident", [128, 128], BF16)
            id1024 = din("id1024", [128, 128], BF16)
            dlhs = din("dlhs", [128, NPAIR, 16], FP8)
            blhs = din("blhs", [H, NPAIR, 128], BF16)
            out_l = dram.tile([BL, 128, KQ, NQ], F32, kind="ExternalOutput",
                              name="out_l", uniquify=False)

            with ExitStack() as body_ctx:
                global _body_ctx
                _body_ctx = body_ctx
                _body(nc, tc, feat8, featr, tok_l, wq, wk, wv, wout,
                      cqs, ck, sk, epsv, nepsv, ident, id1024, dlhs, blhs,
                      out_l)
    nc.compile()
    return nc


def _body(nc, tc, feat8, featr, tok_l, wq, wk, wv, wout, cqs, ck, sk,
          epsv, nepsv, ident, id1024, dlhs, blhs, out_l):
    MULT = mybir.AluOpType.mult
    ADD = mybir.AluOpType.add
    EXP = mybir.ActivationFunctionType.Exp
    COPY = mybir.ActivationFunctionType.Copy

    ctx = _body_ctx
    consts = ctx.enter_context(tc.tile_pool(name="consts", bufs=1))
    kside = ctx.enter_context(tc.tile_pool(name="kside", bufs=1))
    ktmp = ctx.enter_context(tc.tile_pool(name="ktmp", bufs=1))
    featp = ctx.enter_context(tc.tile_pool(name="featp", bufs=2))
    qp = ctx.enter_context(tc.tile_pool(name="qp", bufs=1))
    qsp = ctx.enter_context(tc.tile_pool(name="qsp", bufs=2))
    ep = ctx.enter_context(tc.tile_pool(name="ep", bufs=2))
    atp = ctx.enter_context(tc.tile_pool(name="atp", bufs=2))
    outp = ctx.enter_context(tc.tile_pool(name="outp", bufs=1))
    rp = ctx.enter_context(tc.tile_pool(name="rp", bufs=2))

    pp = ctx.enter_context(tc.tile_pool(name="pp", bufs=2, space="PSUM"))
    attn = ctx.enter_context(tc.tile_pool(name="attn", bufs=4, space="PSUM"))
    dp = ctx.enter_context(tc.tile_pool(name="dp", bufs=2, space="PSUM"))

    st = {}

    # block-diagonalized RoPE'd k: [128, pair, batch, {kA,kB}, 128] bf16
    kbd = kside.tile([128, NPAIR, BL, 2, 128], BF16)
    # block-diagonalized v: [128, batch, pair, 128] fp8
    vbd = kside.tile([128, BL, NPAIR, 128], FP8)
    # zero the off-diagonal quadrants (cheap int32-bitcast DVE memsets)
    nc.vector.memset(kbd[:].bitcast(I32), 0)
    nc.vector.memset(vbd[:].bitcast(I32), 0)

    # ---- constant DMAs.  sync: q-side critical path; scalar: k-side
    # weights + small consts; gpsimd: k tables (sigma DMAs follow there).
    tok_sb = consts.tile([128, DKV], BF16)
    nc.sync.dma_start(out=tok_sb, in_=tok_l[:])
    id_sb = consts.tile([128, 128], BF16)
    nc.sync.dma_start(out=id_sb, in_=ident[:])
    wq_sb = consts.tile([128, KQ, DQ], FP8)
    nc.sync.dma_start(out=wq_sb, in_=wq[:])
    featb0 = featp.tile([128, KQ, NQ], FP8, tag="featb", name="fb0")
    nc.sync.dma_start(out=featb0, in_=feat8[0, :, :, :])
    st.setdefault(0, {})["featb"] = featb0
    cqs_sb = consts.tile([128, 2, NQ], BF16)
    nc.sync.dma_start(out=cqs_sb, in_=cqs[:])
    featb1 = featp.tile([128, KQ, NQ], FP8, tag="featb", name="fb1")
    nc.sync.dma_start(out=featb1, in_=feat8[1, :, :, :])
    st.setdefault(1, {})["featb"] = featb1
    blhs_sb = consts.tile([H, NPAIR, 128], BF16)
    nc.sync.dma_start(out=blhs_sb, in_=blhs[:])
    wout_sb = consts.tile([128, KQ, DQ], FP8)
    nc.sync.dma_start(out=wout_sb, in_=wout[:])
    id1024_sb = consts.tile([128, 128], BF16)
    nc.sync.dma_start(out=id1024_sb, in_=id1024[:])

    wk_sb = consts.tile([128, KKV, DQ], FP8)
    nc.scalar.dma_start(out=wk_sb, in_=wk[:])
    wv_sb = consts.tile([128, KKV, DQ], FP8)
    nc.scalar.dma_start(out=wv_sb, in_=wv[:])
    eps_sb = consts.tile([128, 1], F32)
    nc.scalar.dma_start(out=eps_sb, in_=epsv[:])
    neps_sb = consts.tile([128, 1], F32)
    nc.scalar.dma_start(out=neps_sb, in_=nepsv[:])
    dlhs_sb = consts.tile([128, NPAIR, 16], FP8)
    nc.scalar.dma_start(out=dlhs_sb, in_=dlhs[:])

    ck_sb = consts.tile([128, KQ * 128], BF16)
    nc.gpsimd.dma_start(out=ck_sb, in_=ck[:])
    sk_sb = consts.tile([128, KQ * 128], BF16)
    nc.gpsimd.dma_start(out=sk_sb, in_=sk[:])

    # ---- phase 0a: tokensT + kT (earliest possible PE work)
    _s = nc.named_scope("ph0a")
    _s.__enter__()
    tokT_sb = kside.tile([128, KKV, 128], FP8)
    for ct in range(KKV):
        tp = pp.tile([128, 128], BF16, tag="pp")
        nc.tensor.transpose(tp, tok_sb[:, ct * 128:(ct + 1) * 128], id_sb[:])
        nc.scalar.copy(out=tokT_sb[:, ct, :], in_=tp)
    kT_sb = kside.tile([128, KQ, 128], F32)
    for m in range(KQ):
        kp = pp.tile([128, 128], F32, tag="pp")
        for kc in range(KKV):
            nc.tensor.matmul(kp, wk_sb[:, kc, m * 128:(m + 1) * 128],
                             tokT_sb[:, kc, :],
                             start=(kc == 0), stop=(kc == KKV - 1))
        nc.scalar.copy(out=kT_sb[:, m, :], in_=kp)
    _s.__exit__(None, None, None)

    def phase0_krope():
        t1 = ktmp.tile([128, KQ * 128], F32, tag="t1")
        t2 = ktmp.tile([128, KQ * 128], F32, tag="t2")
        t1s = ktmp.tile([128, KQ * 128], F32, tag="t1s")
        t2s = ktmp.tile([128, KQ * 128], F32, tag="t2s")
        nc.vector.tensor_mul(t1, kT_sb[:], ck_sb[:])
        nc.vector.tensor_mul(t2, kT_sb[:], sk_sb[:])
        _sigma_dma(nc, t1s, t1)
        _sigma_dma(nc, t2s, t2)
        for lo in range(2):
            sl = slice(64 * lo, 64 * lo + 64)
            cl = slice(64 * lo, 64 * lo + 64)
            v3 = lambda t: t[sl, :].rearrange(
                "p (j b t) -> p j b t", j=NPAIR, b=BL)
            nc.vector.scalar_tensor_tensor(
                out=kbd[sl, :, :, 0, cl], in0=v3(t2s),
                scalar=eps_sb[sl], in1=v3(t1), op0=MULT, op1=ADD)
            nc.vector.scalar_tensor_tensor(
                out=kbd[sl, :, :, 1, cl], in0=v3(t1s),
                scalar=neps_sb[sl], in1=v3(t2), op0=MULT, op1=ADD)

    def phase0_v():
        for b in range(BL):
            for (nn0, nsz) in ((0, 512), (512, 256)):
                vp = pp.tile([128, 512], F32, tag="pp")
                for half in range(2):
                    for kc in range(KKV):
                        nc.tensor.matmul(
                            vp[64 * half:64 * half + 64, :nsz],
                            tokT_sb[:, kc, b * 64:(b + 1) * 64],
                            wv_sb[:, kc, nn0:nn0 + nsz],
                            start=(kc == 0), stop=(kc == KKV - 1))
                npair_blk = nsz // 128
                j0 = nn0 // 128
                for half in range(2):
                    sl = slice(64 * half, 64 * half + 64)
                    cl = slice(64 * half, 64 * half + 64)
                    nc.scalar.copy(
                        out=vbd[sl, b, j0:j0 + npair_blk, cl],
                        in_=vp[sl, :nsz].rearrange(
                            "p (j two d) -> p j two d", two=2, d=64
                        )[:, :, half, :])

    # ---- super-chunk stages.  b = batch; each runs chunks c=0,1 paired.

    def qproj(b, ms):
        """qproj m-tiles in `ms` for both chunks of batch b."""
        s = st.setdefault(b, {})
        if "q" not in s:
            if "featb" not in s:
                featb = featp.tile([128, KQ, NQ], FP8, tag="featb",
                                   name=f"fb{b}")
                nc.sync.dma_start(out=featb, in_=feat8[b, :, :, :])
                s["featb"] = featb
            s["q"] = qp.tile([128, KQ, 2, NQ], BF16, tag="q", name=f"q{b}")
        featb, q_sb = s["featb"], s["q"]
        for m in ms:
            qps = {}
            for c in range(NCH):
                qps[c] = pp.tile([128, CHUNK], F32, tag="pp",
                                 name=f"qp{b}_{m}_{c}")
            for kk in range(KQ // 2):
                for c in range(NCH):
                    nc.tensor.matmul(
                        qps[c],
                        wq_sb[:, 2 * kk:2 * kk + 2, m * 128:(m + 1) * 128],
                        featb[:, 2 * kk:2 * kk + 2,
                              c * CHUNK:(c + 1) * CHUNK],
                        start=(kk == 0), stop=(kk == KQ // 2 - 1),
                        perf_mode=DR)
            for c in range(NCH):
                p0 = c * CHUNK
                qstage = qsp.tile([128, CHUNK], BF16, tag="qs", bufs=3,
                                  name=f"qst{b}_{m}_{c}")
                nc.scalar.copy(out=qstage, in_=qps[c])
                nc.vector.tensor_mul(
                    q_sb[:, m, :, p0:p0 + CHUNK],
                    qstage[:, :].unsqueeze(1).broadcast_to([128, 2, CHUNK]),
                    cqs_sb[:, :, p0:p0 + CHUNK])

    def scores(b, js):
        s = st[b]
        q_sb = s["q"]
        if "e" not in s:
            s["e"] = [ep.tile([128, NPAIR, CHUNK], FP8, tag="e",
                              name=f"e{b}_{c}") for c in range(NCH)]
            s["sps"] = {}
        for j in js:
            for c in range(NCH):
                s["sps"][(j, c)] = attn.tile([128, CHUNK], F32, tag="attn",
                                             name=f"s{b}_{j}_{c}")
            for ab in range(2):  # kA then kB (accumulate)
                for c in range(NCH):
                    nc.tensor.matmul(
                        s["sps"][(j, c)], kbd[:, j, b, ab, :],
                        q_sb[:, j, ab, c * CHUNK:(c + 1) * CHUNK],
                        start=(ab == 0), stop=(ab == 1))
            for c in range(NCH):
                nc.scalar.activation(out=s["e"][c][:, j, :],
                                     in_=s["sps"][(j, c)], func=EXP,
                                     scale=PSCALE)

    def denom(b, u):
        s = st[b]
        if "dps" not in s:
            s["dps"] = [dp.tile([16, CHUNK], F32, tag="den",
                                name=f"d{b}_{c}") for c in range(NCH)]
        for c in range(NCH):
            nc.tensor.matmul(s["dps"][c], dlhs_sb[:, 2 * u:2 * u + 2, :],
                             s["e"][c][:, 2 * u:2 * u + 2, :],
                             start=(u == 0), stop=(u == NPAIR // 2 - 1),
                             perf_mode=DR)

    def recip(b):
        s = st[b]
        s["r"] = []
        for c in range(NCH):
            r32 = rp.tile([16, CHUNK], F32, tag="r32", name=f"r32_{b}_{c}")
            nc.vector.reciprocal_approx_fast(out=r32, in_=s["dps"][c])
            r_sb = rp.tile([H, CHUNK], BF16, tag="r", name=f"r{b}_{c}")
            nc.vector.tensor_copy(r_sb, r32[:H, :])
            s["r"].append(r_sb)

    def avbc(b, js):
        s = st[b]
        if "attnT" not in s:
            s["attnT"] = [atp.tile([128, NPAIR, CHUNK], FP8, tag="attnT",
                                   name=f"at{b}_{c}") for c in range(NCH)]
        for j in js:
            aps, bps = {}, {}
            for c in range(NCH):
                aps[c] = attn.tile([128, CHUNK], F32, tag="attn",
                                   name=f"a{b}_{j}_{c}")
                nc.tensor.matmul(aps[c], vbd[:, b, j, :], s["e"][c][:, j, :],
                                 start=True, stop=True)
            for c in range(NCH):
                bps[c] = attn.tile([128, CHUNK], F32, tag="attn",
                                   name=f"b{b}_{j}_{c}")
                nc.tensor.matmul(bps[c], blhs_sb[:, j, :], s["r"][c][:],
                                 start=True, stop=True)
            for c in range(NCH):
                avs = rp.tile([128, CHUNK], BF16, tag="avs", bufs=4,
                              name=f"avs{b}_{j}_{c}")
                nc.scalar.copy(out=avs, in_=aps[c])
                nc.vector.tensor_mul(s["attnT"][c][:, j, :], avs, bps[c])

    def oproj(b, ms, den_next=None):
        s = st[b]
        featrb, attnT = s["featrb"], s["attnT"]
        if "o" not in s:
            s["o"] = outp.tile([128, KQ, NQ], F32, tag="osb", name=f"o{b}")
        o_sb = s["o"]
        for mi, m in enumerate(ms):
            ops = {}
            for c in range(NCH):
                ops[c] = pp.tile([128, CHUNK], F32, tag="pp",
                                 name=f"op{b}_{m}_{c}")
            for kk in range(KQ // 2):
                for c in range(NCH):
                    nc.tensor.matmul(
                        ops[c],
                        wout_sb[:, 2 * kk:2 * kk + 2, m * 128:(m + 1) * 128],
                        attnT[c][:, 2 * kk:2 * kk + 2, :],
                        start=(kk == 0), stop=False, perf_mode=DR)
            for c in range(NCH):
                nc.tensor.matmul(ops[c], id1024_sb[:],
                                 featrb[:, m, c * CHUNK:(c + 1) * CHUNK],
                                 start=False, stop=True)
            if den_next is not None and m % 2 == 1:
                denom(den_next, m // 2)
            for c in range(NCH):
                nc.scalar.activation(out=o_sb[:, m, c * CHUNK:(c + 1) * CHUNK],
                                     in_=ops[c], func=COPY, scale=PSCALE)
            if m in (1, 3):
                nc.sync.dma_start(out=out_l[b, :, m - 1:m + 1, :],
                                  in_=o_sb[:, m - 1:m + 1, :])
            elif m >= 4:
                nc.sync.dma_start(out=out_l[b, :, m, :], in_=o_sb[:, m, :])

    def load_featr(b):
        s = st.setdefault(b, {})
        featrb = featp.tile([128, KQ, NQ], BF16, tag="featr", name=f"fr{b}")
        nc.scalar.dma_start(out=featrb, in_=featr[b, :, :, :])
        s["featrb"] = featrb

    def sc(tag):
        return nc.named_scope(tag)

    # ---- emission schedule (PE program order)
    with sc("ph0k"):
        phase0_krope()
    with sc("qp0"):
        load_featr(0)
        qproj(0, range(KQ))
    with sc("ph0v"):
        phase0_v()
    with sc("qk0"):
        scores(0, range(NPAIR))
    with sc("dn0"):
        for u in range(NPAIR // 2):
            denom(0, u)
    with sc("rc0"):
        recip(0)
    with sc("qp1"):
        load_featr(1)
        qproj(1, range(KQ))
    with sc("av0"):
        avbc(0, range(NPAIR))
    with sc("qk1"):
        scores(1, range(NPAIR))
    with sc("op0"):
        oproj(0, range(KQ), den_next=1)
    with sc("rc1"):
        recip(1)
    with sc("av1"):
        avbc(1, range(NPAIR))
    with sc("op1"):
        oproj(1, range(KQ))


_NC_CACHE = {}


def _get_nc():
    if "nc" not in _NC_CACHE:
        _NC_CACHE["nc"] = build(debug=False)
    return _NC_CACHE["nc"]


def _prep_in_maps(feat, tokens, Wq, Wkv, Wout, bout):
    feat = np.ascontiguousarray(feat, dtype=np.float32).reshape(B, DQ, NQ)
    tokens = np.ascontiguousarray(tokens, dtype=np.float32)
    featb = feat + bout.astype(np.float32)[None, :, None]
    shared = dict(
        wq=np.ascontiguousarray(
            (Wq * WS).reshape(KQ, 128, DQ).transpose(1, 0, 2), dtype=NPF8),
        wk=np.ascontiguousarray(
            (Wkv[:, :DQ] * WS).reshape(KKV, 128, DQ).transpose(1, 0, 2),
            dtype=NPF8),
        wv=np.ascontiguousarray(
            (Wkv[:, DQ:] * WS).reshape(KKV, 128, DQ).transpose(1, 0, 2),
            dtype=NPF8),
        wout=np.ascontiguousarray(
            (Wout * WS).reshape(KQ, 128, DQ).transpose(1, 0, 2), dtype=NPF8),
        **_consts(),
    )
    in_maps = []
    for cid in range(NCORES):
        sl = slice(BL * cid, BL * (cid + 1))
        fl = feat[sl].reshape(BL, KQ, 128, NQ).transpose(0, 2, 1, 3)
        fr = featb[sl].reshape(BL, KQ, 128, NQ).transpose(0, 2, 1, 3)
        tl = np.ascontiguousarray(tokens[sl].reshape(BL * NT, DKV), dtype=NPBF)
        in_maps.append(dict(
            feat8=np.ascontiguousarray(fl, dtype=NPF8),
            featr=np.ascontiguousarray(fr, dtype=NPBF),
            tok_l=tl, **shared))
    return in_maps


def _install_ntff_hook():
    import types

    import antenv
    from trn_agent_boot.trn_boot import _ntff_profile_via_ctypes

    mod = types.ModuleType("antenv.axon_hooks")
    state = {"hook": None}
    mod.set_axon_ntff_profile_hook = lambda h: state.__setitem__("hook", h)
    mod.get_axon_ntff_profile_hook = lambda: state["hook"]
    sys.modules["antenv.axon_hooks"] = mod
    antenv.axon_hooks = mod
    mod.set_axon_ntff_profile_hook(
        _ntff_profile_via_ctypes("/opt/axon/libaxon_pjrt.so"))
    import concourse.bass_utils as bu
    bu.upload_artifacts = lambda tmpdir: f"local:{tmpdir}"


def run(inputs, trace=False, trace_cores=None):
    nc = _get_nc()
    if trace:
        try:
            _install_ntff_hook()
        except Exception as e:
            print(f"ntff hook install failed: {e}", file=sys.stderr)
            trace = False
    in_maps = _prep_in_maps(**inputs)
    res = run_bass_kernel_spmd(nc, in_maps, core_ids=list(range(NCORES)),
                               trace=trace, trace_cores=trace_cores)
    outs = []
    for r in res.results:
        ol = r["out_l"]  # [BL, 128, KQ, NQ]
        outs.append(ol.transpose(0, 2, 1, 3).reshape(BL, DQ, T, HP, WP))
    return np.ascontiguousarray(np.concatenate(outs, axis=0)), res


def kernel(**inputs):
    return run(inputs, trace=False)[0]
